# revision 9
# baseline (speedup 1.0000x reference)
"""KANLinear TRN2 Bass kernel (8-core SPMD, token-data-parallel, fp8 DR).

Math (exact tent identity, verified to 7e-16 vs the jax reference):
  with s = 4x + 7 and tents m_j(s) = relu(min(s-j, j+2-s)) = relu(1-|s-j-1|),
  6*B_g(x) = m3_g - 2*m3_{g+1} + m3_{g+2} + 6*m_{g+1}   (m3 = m^3, g=0..10)
All values in [0,4]; no catastrophic cancellation anywhere, no x-clamp
needed (features vanish outside the grid automatically, like the reference).

Per 512-token half, per in-dim tile (128 rows):
  ACT : a_j = |4x + (6-j)| for j=0..12; silu
  DVE : negm = min(a,1)-1 = -m (ts 4x); sqA = negm*negm (tt 2x)
        negc = sqA*negm = -m^3 (tt 2x); ncDn = -2*negc, Q6p = 6*negm (ts 4x)
        u = 2*sqA-6 (ts); P2 = u*negm (tt); X = negc_g + negc_{g+2} (tt)
        FEAT = -6*B_g = X - P2 -> fp8 slabs (8 direct on DVE, 3 via ACT Copy)
The 11 fp8 feature slabs per in-dim feed a DoubleRow fp8 matmul (measured
216ns per K=256/N=512 MM = 2x bf16); host weights are -coeff*ss/6, DR-paired.
The silu base matmul stays bf16. Tokens sharded 1024/core; weights replicated.
"""

import numpy as np
import ml_dtypes

import concourse.bass as bass
import concourse.mybir as mybir
import concourse.tile as tile
from concourse import bacc
from concourse.alu_op_type import AluOpType
from concourse.bass_utils import run_bass_kernel_spmd

AF = mybir.ActivationFunctionType
F32 = mybir.dt.float32
F16 = mybir.dt.float16
BF16 = mybir.dt.bfloat16
FP8 = mybir.dt.float8e4
DR = mybir.MatmulPerfMode.DoubleRow

TOKENS, IN_DIM, OUT_DIM = 8192, 1024, 1024
N_CORES = 8
TPC = TOKENS // N_CORES      # 1024 tokens per core
HALF = 512                   # tokens per processing half (PSUM-limited)
NIT = IN_DIM // 128          # 8 in-dim tiles
NCH = 13                     # tent channels j = 0..12
NB = 11                      # bases g = 0..10
NACT = 13                    # abs channels on ACT
M_TILES = HALF // 128        # 4
N_OC = OUT_DIM // 512        # 2
NKT = NIT * NB               # 88 spline K-tiles per half
NQ = NKT // 2                # 44 DoubleRow K-pairs
CW = NCH * HALF              # 6656
BW = NB * HALF               # 5632

_CACHED = None


def _build_bass():
    nc = bacc.Bacc("TRN2", target_bir_lowering=False, debug=False,
                   num_devices=N_CORES)
    xt = nc.declare_dram_parameter("xt", [IN_DIM, TPC], F32, isOutput=False)
    w8 = nc.declare_dram_parameter("w8", [NQ, 128, 2, OUT_DIM], FP8,
                                   isOutput=False)
    sbt = nc.declare_dram_parameter("sbt", [IN_DIM, OUT_DIM], BF16,
                                    isOutput=False)
    y = nc.declare_dram_parameter("y", [TPC, OUT_DIM], F32, isOutput=True)

    with tile.TileContext(nc) as tc:
        with (
            tc.tile_pool(name="consts", bufs=1) as kpool,
            tc.tile_pool(name="xts", bufs=3) as xpool,
            tc.tile_pool(name="silu", bufs=9) as spool,
            tc.tile_pool(name="sbtp", bufs=8) as sbpool,
            tc.tile_pool(name="abuf", bufs=2) as apool,
            tc.tile_pool(name="ew1", bufs=1) as e1,
            tc.tile_pool(name="ew2", bufs=1) as e2,
            tc.tile_pool(name="fbp", bufs=2) as fbp,
            tc.tile_pool(name="feat", bufs=1) as fpool,
            tc.tile_pool(name="wts", bufs=4) as wpool,
            tc.tile_pool(name="outs", bufs=3) as opool,
            tc.tile_pool(name="psum", bufs=8, space="PSUM") as ppool,
        ):
            bias = kpool.tile([128, NACT], F32, tag="bias")
            for j in range(NACT):
                nc.vector.memset(bias[:, j:j + 1], float(6 - j))

            sbt_tiles = []
            for it in range(NIT):
                t = sbpool.tile([128, OUT_DIM], BF16, tag="sbt")
                nc.sync.dma_start(out=t, in_=sbt[it * 128:(it + 1) * 128, :])
                sbt_tiles.append(t)

            for half in range(2):
                t0 = half * HALF
                feat = fpool.tile([128, NKT, HALF], FP8, tag="feat",
                                  name=f"feat_{half}")
                psums = [[ppool.tile([128, 512], F32, tag="ps",
                                     name=f"ps_{half}_{oc}_{m}")
                          for m in range(M_TILES)] for oc in range(N_OC)]
                silu_tiles = []
                for it in range(NIT):
                    xtt = xpool.tile([128, HALF], F32, tag="xt")
                    nc.sync.dma_start(
                        out=xtt, in_=xt[it * 128:(it + 1) * 128, t0:t0 + HALF])
                    st = spool.tile([128, HALF], BF16, tag="silu")
                    nc.scalar.activation(st, xtt, AF.Silu)
                    silu_tiles.append(st)

                    # 13 tent-distance channels a_j = |4x + (6-j)| (fp16),
                    # split across ACT (j<NACT) and DVE (rest, from xs)
                    a = apool.tile([128, CW], F16, tag="a")
                    for j in range(NACT):
                        nc.scalar.activation(
                            a[:, j * HALF:(j + 1) * HALF], xtt, AF.Abs,
                            bias=bias[:, j:j + 1], scale=4.0)
                    # negm = min(a,1) - 1 = -m
                    negm = e1.tile([128, CW], F16, tag="negm")
                    nc.vector.tensor_scalar(
                        out=negm, in0=a, scalar1=1.0, scalar2=1.0,
                        op0=AluOpType.min, op1=AluOpType.subtract)
                    # negc = negm^2 * negm = -m^3
                    sqA = e1.tile([128, CW], F16, tag="sqA")
                    nc.scalar.activation(sqA, negm, AF.Square)
                    negc = e1.tile([128, CW], F16, tag="negc")
                    nc.vector.tensor_tensor(out=negc, in0=sqA, in1=negm,
                                            op=AluOpType.mult)
                    # u = 2*m^2 - 6; P2 = u*negm = 2m^3... = -(2m^3 - 6m)
                    u = e2.tile([128, BW], F16, tag="u")
                    nc.vector.tensor_scalar(
                        out=u, in0=sqA[:, HALF:HALF + BW], scalar1=3.0,
                        scalar2=2.0, op0=AluOpType.subtract, op1=AluOpType.mult)
                    P2 = e2.tile([128, BW], F16, tag="P2")
                    nc.vector.tensor_tensor(out=P2, in0=u, in1=negm[:, HALF:
                                                                    HALF + BW],
                                            op=AluOpType.mult)
                    # FEAT = -6*B_g = X - P2
                    X = e2.tile([128, BW], F16, tag="X")
                    nc.vector.tensor_tensor(
                        out=X, in0=negc[:, 0:BW],
                        in1=negc[:, 2 * HALF:2 * HALF + BW],
                        op=AluOpType.add)
                    KD = 8 * HALF  # slabs 0..7 direct fp8 (DVE 1x)
                    nc.vector.tensor_tensor(
                        out=feat[:, it * NB:it * NB + 8, :],
                        in0=X[:, 0:KD], in1=P2[:, 0:KD],
                        op=AluOpType.subtract)
                    Fb = fbp.tile([128, BW - KD], F16, tag="Fb")
                    nc.vector.tensor_tensor(
                        out=Fb, in0=X[:, KD:BW], in1=P2[:, KD:BW],
                        op=AluOpType.subtract)
                    nc.scalar.activation(
                        feat[:, it * NB + 8:(it + 1) * NB, :], Fb, AF.Copy)

                # base matmul (bf16) opens the PSUM accumulation groups
                for it in range(NIT):
                    for oc in range(N_OC):
                        for m in range(M_TILES):
                            nc.tensor.matmul(
                                psums[oc][m],
                                lhsT=silu_tiles[it][:, m * 128:m * 128 + 128],
                                rhs=sbt_tiles[it][:, oc * 512:(oc + 1) * 512],
                                start=(it == 0), stop=False)
                # spline matmul: fp8 DoubleRow over 44 K-pairs
                for q in range(NQ):
                    wt = wpool.tile([128, 2, OUT_DIM], FP8, tag="w8")
                    nc.sync.dma_start(out=wt, in_=w8[q, :, :, :])
                    for oc in range(N_OC):
                        for m in range(M_TILES):
                            nc.tensor.matmul(
                                psums[oc][m],
                                lhsT=feat[:, 2 * q:2 * q + 2,
                                          m * 128:m * 128 + 128],
                                rhs=wt[:, :, oc * 512:(oc + 1) * 512],
                                start=False, stop=(q == NQ - 1),
                                perf_mode=DR)
                # drain
                for oc in range(N_OC):
                    for m in range(M_TILES):
                        ot = opool.tile([128, 512], F32, tag="out")
                        nc.scalar.copy(ot, psums[oc][m])
                        r0 = t0 + m * 128
                        nc.sync.dma_start(
                            out=y[r0:r0 + 128, oc * 512:(oc + 1) * 512],
                            in_=ot)
    nc.compile()
    return nc


def _prepare_inputs(x, coeff, scale_base, scale_spline):
    x = np.asarray(x, dtype=np.float32)
    coeff = np.asarray(coeff, dtype=np.float32)
    scale_base = np.asarray(scale_base, dtype=np.float32)
    ss = float(np.asarray(scale_spline).reshape(-1)[0])
    # features are -6*B_g -> weights are -coeff*ss/6
    # K-row kt = it*11 + g; DR-paired: w8[q, p, r, o] = w[kt=2q+r][p, o]
    w = (coeff * (-ss / 6.0)).transpose(1, 2, 0).reshape(
        NIT, 128, NB, OUT_DIM).transpose(0, 2, 1, 3)  # [it, g, p, o]
    w8 = np.ascontiguousarray(
        w.reshape(NKT, 128, OUT_DIM).reshape(NQ, 2, 128, OUT_DIM)
        .transpose(0, 2, 1, 3)).astype(ml_dtypes.float8_e4m3fn)
    sbt = np.ascontiguousarray(scale_base.T).astype(ml_dtypes.bfloat16)
    in_maps = []
    for c in range(N_CORES):
        xtc = np.ascontiguousarray(x[c * TPC:(c + 1) * TPC, :].T)
        in_maps.append({"xt": xtc, "w8": w8, "sbt": sbt})
    return in_maps


def _get_bass():
    global _CACHED
    if _CACHED is None:
        _CACHED = _build_bass()
    return _CACHED


def run(inputs, trace=False, **kw):
    nc = _get_bass()
    in_maps = _prepare_inputs(inputs["x"], inputs["coeff"],
                              inputs["scale_base"], inputs["scale_spline"])
    res = run_bass_kernel_spmd(nc, in_maps, list(range(N_CORES)),
                               trace=trace, **kw)
    yv = np.concatenate([np.asarray(res.results[c]["y"])
                         for c in range(N_CORES)], axis=0)
    return np.ascontiguousarray(yv.astype(np.float32)), res


def kernel(x, grid, coeff, scale_base, scale_spline):
    yv, _ = run({"x": x, "grid": grid, "coeff": coeff,
                 "scale_base": scale_base, "scale_spline": scale_spline})
    return yv
